# revision 6
# baseline (speedup 1.0000x reference)
"""Causal self-attention (sparse column mask) on 8 Trainium2 NeuronCores.

Problem: B=8, T=1024, C=512, 8 heads (hd=64).
  q/k/v = x @ W{q,k,v}.T + b;  att = softmax(mask(q k^T / 8));  y = att v
  out = y @ Wp.T + bp
Mask: causal lower-triangle, minus every column j with j % 25 == 24.

Sharding: pure data-parallel over batch — core b computes batch element b.

Per-core kernel design (all matmul operands fp16, PSUM accumulation f32):
  - Host pre-transposes x[b] -> xT [C, T] and all weights -> W^T [c_in, c_out],
    so every on-chip matmul has its contraction dim on partitions.
  - Projections produce q^T, k^T [C, T] (heads = partition blocks of 64) and
    v [T, C]. q bias is added via a K=1 matmul (ones rhs); k bias is dropped
    (softmax shift invariance); v bias is folded into the output bias on host
    (bp' = Wp @ bv + bp).
  - Attention per head-pair p (heads 2p, 2p+1 live in partitions 0:64 / 64:128
    of q^T/k^T tile p), per query chunk ic (512 wide), per key tile J (128):
      S^T[j, i] via two row-tiled (tile_position) K=64 matmuls -> 2 PSUM banks
      exp via one ACT call (scale=1/8, per-partition bias = -30 on masked
      columns j%25==24) -> fp16 SBUF
      causal diagonal handled by multiplying the diagonal 128x128 block of
      exp(S^T) with a 0/1 lower-triangle fp16 tile on DVE
      y'^T accumulation: two col-tiled matmuls (v as stationary) into one bank
      denominators: two col-tiled M=64 ones-matmuls -> den replicated across
      each head's 64 partitions, lane-aligned with y'^T
    After the J loop: rden = approx reciprocal (DVE), y_norm^T = y' * rden
    (one tensor_tensor multiply, PSUM -> SBUF fp16).
  - Output projection consumes y_norm^T directly (contraction on partitions),
    bias via K=1 matmul, result DMA'd from PSUM to DRAM in f32.
"""

import math

import numpy as np

B, T, C = 8, 1024, 512
H = 8
HD = C // H
P = 128
JD = 25  # joined dim; column j masked when j % 25 == 24
N_CORES = 8
NEG = -30.0  # added post-scale; exp(-30) flushes to 0 in fp16

_CACHE = {}


def _build():
    import concourse.mybir as mybir
    import concourse.tile as tile
    from concourse import bacc

    f16 = mybir.dt.float16
    f32 = mybir.dt.float32
    AF = mybir.ActivationFunctionType

    nc = bacc.Bacc("TRN2", target_bir_lowering=False, debug=False)

    xT = nc.dram_tensor("xT", [C, T], f16, kind="ExternalInput").ap()
    wqT = nc.dram_tensor("wqT", [C, C], f16, kind="ExternalInput").ap()
    wkT = nc.dram_tensor("wkT", [C, C], f16, kind="ExternalInput").ap()
    wvT = nc.dram_tensor("wvT", [C, C], f16, kind="ExternalInput").ap()
    wpT = nc.dram_tensor("wpT", [C, C], f16, kind="ExternalInput").ap()
    bq = nc.dram_tensor("bq", [1, C], f16, kind="ExternalInput").ap()
    bpp = nc.dram_tensor("bpp", [1, C], f16, kind="ExternalInput").ap()
    ones_row = nc.dram_tensor("ones_row", [1, C], f16, kind="ExternalInput").ap()
    ones64 = nc.dram_tensor("ones64", [P, HD], f16, kind="ExternalInput").ap()
    tri = nc.dram_tensor("tri", [P, P], f16, kind="ExternalInput").ap()
    cmask = nc.dram_tensor("cmask", [P, T // P], f32, kind="ExternalInput").ap()
    out = nc.dram_tensor("out", [T, C], f32, kind="ExternalOutput").ap()

    KT = C // P  # 4 c_in tiles
    MT = C // P  # 4 c_out tiles (= head pairs)
    RT = T // P  # 8 t tiles

    with tile.TileContext(nc) as tc:
        with (
            tc.tile_pool(name="const", bufs=1) as const,
            tc.tile_pool(name="persist", bufs=1) as persist,
            tc.tile_pool(name="es", bufs=3) as es_pool,
            tc.tile_pool(name="rden", bufs=2) as rden_pool,
            tc.tile_pool(name="pbig", bufs=2, space="PSUM") as pbig,
            tc.tile_pool(name="psmall", bufs=4, space="PSUM") as psmall,
        ):
            # ---- load constants / inputs ----
            def load(pool, shape, dtype, src, tag):
                t = pool.tile(shape, dtype, name=tag, tag=tag)
                nc.sync.dma_start(out=t, in_=src)
                return t

            xT_t = [
                load(const, [P, T], f16, xT[P * k : P * (k + 1), :], f"xT{k}")
                for k in range(KT)
            ]
            wq_t = [
                load(const, [P, C], f16, wqT[P * k : P * (k + 1), :], f"wq{k}")
                for k in range(KT)
            ]
            wk_t = [
                load(const, [P, C], f16, wkT[P * k : P * (k + 1), :], f"wk{k}")
                for k in range(KT)
            ]
            wv_t = [
                load(const, [P, C], f16, wvT[P * k : P * (k + 1), :], f"wv{k}")
                for k in range(KT)
            ]
            wp_t = [
                load(const, [P, C], f16, wpT[P * k : P * (k + 1), :], f"wp{k}")
                for k in range(KT)
            ]
            bq_s = load(const, [1, C], f16, bq, "bq")
            bpp_s = load(const, [1, C], f16, bpp, "bpp")
            onesr_s = load(const, [1, C], f16, ones_row, "onesr")
            ones64_s = load(const, [P, HD], f16, ones64, "ones64")
            tri_s = load(const, [P, P], f16, tri, "tri")
            cmask_s = load(const, [P, T // P], f32, cmask, "cmask")

            qT_t = [persist.tile([P, T], f16, name=f"qT{m}", tag=f"qT{m}") for m in range(MT)]
            kT_t = [persist.tile([P, T], f16, name=f"kT{m}", tag=f"kT{m}") for m in range(MT)]
            v_t = [persist.tile([P, C], f16, name=f"v{r}", tag=f"v{r}") for r in range(RT)]
            yn_t = [persist.tile([P, T], f16, name=f"yn{m}", tag=f"yn{m}") for m in range(MT)]

            # ---- projections ----
            # q^T / k^T: out[co, t] = sum_c W^T[c, co] * xT[c, t] (+ bias for q)
            for m in range(MT):
                for w_t, dst, biased in ((wq_t, qT_t[m], True), (wk_t, kT_t[m], False)):
                    ps = pbig.tile([P, T], mybir.dt.float32, tag="pbig")
                    for half in range(2):
                        o = ps[:, 512 * half : 512 * (half + 1)]
                        if biased:
                            nc.tensor.matmul(
                                o,
                                lhsT=bq_s[:, P * m : P * (m + 1)],
                                rhs=onesr_s[:, :512],
                                start=True,
                                stop=False,
                            )
                        for k in range(KT):
                            nc.tensor.matmul(
                                o,
                                lhsT=w_t[k][:, P * m : P * (m + 1)],
                                rhs=xT_t[k][:, 512 * half : 512 * (half + 1)],
                                start=(k == 0 and not biased),
                                stop=(k == KT - 1),
                            )
                    nc.scalar.activation(dst, ps, AF.Copy)
            # v: out[t, co] = sum_c xT[c, t] * wvT[c, co]
            for r in range(RT):
                ps = psmall.tile([P, C], mybir.dt.float32, name="pv", tag="sm")
                for k in range(KT):
                    nc.tensor.matmul(
                        ps,
                        lhsT=xT_t[k][:, P * r : P * (r + 1)],
                        rhs=wv_t[k],
                        start=(k == 0),
                        stop=(k == KT - 1),
                    )
                nc.vector.tensor_copy(v_t[r], ps)

            # ---- attention ----
            for ic in range(2):
                for p in range(MT):
                    av = psmall.tile([P, 512], mybir.dt.float32, name="av", tag="sm")
                    den = psmall.tile([P, 512], mybir.dt.float32, name="den", tag="sm")
                    Js = list(range(4 * (ic + 1)))
                    for idx, J in enumerate(Js):
                        i0 = max(512 * ic, P * J)
                        w = 512 * (ic + 1) - i0
                        io = i0 - 512 * ic
                        first, last = idx == 0, idx == len(Js) - 1
                        st = pbig.tile([P, 2, 512], mybir.dt.float32, name="st", tag="pbig")
                        # S^T: two row-tiled K=64 matmuls (one per head)
                        for h in range(2):
                            nc.tensor.matmul(
                                st[:, h, :w],
                                lhsT=kT_t[p][64 * h : 64 * (h + 1), P * J : P * (J + 1)],
                                rhs=qT_t[p][64 * h : 64 * (h + 1), i0 : i0 + w],
                                start=True,
                                stop=True,
                                tile_position=(64 * h, 0),
                            )
                        es = es_pool.tile([P, 2, 512], f16, tag="es")
                        nc.scalar.activation(
                            es[:, :, :w],
                            st[:, :, :w],
                            AF.Exp,
                            bias=cmask_s[:, J : J + 1],
                            scale=0.125,
                        )
                        if P * J >= 512 * ic:  # diagonal block: causal triangle
                            for h in range(2):
                                nc.vector.tensor_mul(
                                    es[:, h, :P], es[:, h, :P], tri_s
                                )
                        # y'^T accumulation + denominators (col-tiled pairs)
                        for h in range(2):
                            nc.tensor.matmul(
                                av[64 * h : 64 * (h + 1), io : io + w],
                                lhsT=v_t[J][:, P * p + 64 * h : P * p + 64 * (h + 1)],
                                rhs=es[:, h, :w],
                                start=first,
                                stop=last,
                                tile_position=(0, 64 * h),
                            )
                            nc.tensor.matmul(
                                den[64 * h : 64 * (h + 1), io : io + w],
                                lhsT=ones64_s,
                                rhs=es[:, h, :w],
                                start=first,
                                stop=last,
                                tile_position=(0, 64 * h),
                            )
                    rden = rden_pool.tile([P, 512], mybir.dt.float32, tag="rden")
                    nc.vector.reciprocal_approx_fast(out=rden, in_=den)
                    nc.vector.tensor_mul(
                        yn_t[p][:, 512 * ic : 512 * (ic + 1)], av, rden
                    )

            # ---- output projection ----
            for r in range(RT):
                po = psmall.tile([P, C], mybir.dt.float32, name="po", tag="sm")
                nc.tensor.matmul(
                    po, lhsT=onesr_s[:, :P], rhs=bpp_s, start=True, stop=False
                )
                for m in range(MT):
                    nc.tensor.matmul(
                        po,
                        lhsT=yn_t[m][:, P * r : P * (r + 1)],
                        rhs=wp_t[m],
                        start=False,
                        stop=(m == MT - 1),
                    )
                ot = es_pool.tile([P, C], mybir.dt.float32, name="ot", tag="ot")
                nc.vector.tensor_copy(ot, po)
                nc.sync.dma_start(out=out[P * r : P * (r + 1), :], in_=ot)

    nc.compile()
    return nc


def _prep_inputs(x, Wq, bq, Wk, bk, Wv, bv, Wp, bp):
    """Host-side prep: transposes, bias folding, mask tables. Returns in_maps."""
    f16 = np.float16
    wqT = np.ascontiguousarray(Wq.T).astype(f16)
    wkT = np.ascontiguousarray(Wk.T).astype(f16)
    wvT = np.ascontiguousarray(Wv.T).astype(f16)
    wpT = np.ascontiguousarray(Wp.T).astype(f16)
    bq16 = bq.astype(f16).reshape(1, C)
    # v bias folds into output bias: out = (y' + bv) @ Wp.T + bp
    bpp = (Wp.astype(np.float64) @ bv.astype(np.float64) + bp.astype(np.float64)).astype(
        f16
    ).reshape(1, C)
    ones_row = np.ones((1, C), dtype=f16)
    ones64 = np.ones((P, HD), dtype=f16)
    tri = (np.arange(P)[:, None] <= np.arange(P)[None, :]).astype(f16)  # keep j<=i
    j_idx = np.arange(P)[:, None] + P * np.arange(T // P)[None, :]
    cmask = np.where(j_idx % JD == JD - 1, np.float32(NEG), np.float32(0.0))

    in_maps = []
    for b in range(N_CORES):
        xT = np.ascontiguousarray(x[b].T).astype(f16)
        in_maps.append(
            {
                "xT": xT,
                "wqT": wqT,
                "wkT": wkT,
                "wvT": wvT,
                "wpT": wpT,
                "bq": bq16,
                "bpp": bpp,
                "ones_row": ones_row,
                "ones64": ones64,
                "tri": tri,
                "cmask": cmask.astype(np.float32),
            }
        )
    return in_maps


def kernel(x, Wq, bq, Wk, bk, Wv, bv, Wp, bp):
    from concourse import bass_utils

    x = np.asarray(x, dtype=np.float32)
    if "nc" not in _CACHE:
        _CACHE["nc"] = _build()
    nc = _CACHE["nc"]
    in_maps = _prep_inputs(
        x,
        np.asarray(Wq, np.float32),
        np.asarray(bq, np.float32),
        np.asarray(Wk, np.float32),
        np.asarray(bk, np.float32),
        np.asarray(Wv, np.float32),
        np.asarray(bv, np.float32),
        np.asarray(Wp, np.float32),
        np.asarray(bp, np.float32),
    )
    res = bass_utils.run_bass_kernel_spmd(nc, in_maps, core_ids=list(range(N_CORES)))
    return np.stack([res.results[b]["out"] for b in range(N_CORES)], axis=0)


# revision 7
# speedup vs baseline: 1.1859x; 1.1859x over previous
"""Causal self-attention (sparse column mask) on 8 Trainium2 NeuronCores.

Problem: B=8, T=1024, C=512, 8 heads (hd=64).
  q/k/v = x @ W{q,k,v}.T + b;  att = softmax(mask(q k^T / 8));  y = att v
  out = y @ Wp.T + bp
Mask: causal lower-triangle, minus every column j with j % 25 == 24.

Sharding: pure data-parallel over batch — core b computes batch element b.

Per-core kernel design (all matmul operands fp16, PSUM accumulation f32):
  - Host pre-transposes x[b] -> xT [C, T] and all weights -> W^T [c_in, c_out],
    so every on-chip matmul has its contraction dim on partitions.
  - Projections produce q^T, k^T [C, T] (heads = partition blocks of 64) and
    v [T, C]. q bias is added during PSUM evacuation (DVE tensor_scalar,
    per-partition); k bias is dropped (softmax shift invariance); v bias is
    folded into the output bias on host (bp' = Wp @ bv + bp, sent broadcast).
  - Attention, phase-separated per query chunk ic (512 wide) so the PE array
    stays in one tiling mode per phase (mode switches drain the PE):
      QK phase (64x128 row-tiled): per head-pair p, per key tile J: two K=64
      matmuls (tile_position (0,0)/(64,0)) -> S^T in 2 PSUM banks; one ACT
      exp call over both (scale=1/8, per-partition bias -30 on j%25==24
      columns) -> fp16 SBUF; causal diagonal zeroed by one fp16 multiply with
      a broadcast lower-triangle tile on DVE.
      AV phase (128x64 col-tiled): per pair, accumulate y'^T and the
      replicated denominators (ones-weight matmuls) over J; then rden via
      approx reciprocal and one tensor_tensor multiply PSUM->SBUF fp16.
  - Output projection consumes y_norm^T directly; bias added during the DVE
    evacuation (tensor_tensor add with a host-broadcast bias tile).
"""

import numpy as np

B, T, C = 8, 1024, 512
H = 8
HD = C // H
P = 128
JD = 25  # joined dim; column j masked when j % 25 == 24
N_CORES = 8
NEG = -30.0  # added post-scale; exp(-30) flushes to 0 in fp16

_CACHE = {}


def _build():
    import concourse.bass as bass
    import concourse.mybir as mybir
    import concourse.tile as tile
    from concourse import bacc

    f16 = mybir.dt.float16
    f32 = mybir.dt.float32
    AF = mybir.ActivationFunctionType
    ALU = mybir.AluOpType

    nc = bacc.Bacc("TRN2", target_bir_lowering=False, debug=False)

    xT = nc.dram_tensor("xT", [C, T], f16, kind="ExternalInput").ap()
    wqT = nc.dram_tensor("wqT", [C, C], f16, kind="ExternalInput").ap()
    wkT = nc.dram_tensor("wkT", [C, C], f16, kind="ExternalInput").ap()
    wvT = nc.dram_tensor("wvT", [C, C], f16, kind="ExternalInput").ap()
    wpT = nc.dram_tensor("wpT", [C, C], f16, kind="ExternalInput").ap()
    bq = nc.dram_tensor("bq", [P, C // P], f32, kind="ExternalInput").ap()
    bppb = nc.dram_tensor("bppb", [P, C], f32, kind="ExternalInput").ap()
    ones64 = nc.dram_tensor("ones64", [P, HD], f16, kind="ExternalInput").ap()
    tri = nc.dram_tensor("tri", [P, P], f16, kind="ExternalInput").ap()
    cmask = nc.dram_tensor("cmask", [P, T // P], f32, kind="ExternalInput").ap()
    out = nc.dram_tensor("out", [T, C], f32, kind="ExternalOutput").ap()

    KT = C // P  # 4 c_in tiles
    MT = C // P  # 4 c_out tiles (= head pairs)
    RT = T // P  # 8 t tiles

    with tile.TileContext(nc) as tc:
        with (
            tc.tile_pool(name="const", bufs=1) as const,
            tc.tile_pool(name="persist", bufs=1) as persist,
            tc.tile_pool(name="es", bufs=32) as es_pool,
            tc.tile_pool(name="rden", bufs=2) as rden_pool,
            tc.tile_pool(name="ot", bufs=3) as ot_pool,
            tc.tile_pool(name="pbig", bufs=2, space="PSUM") as pbig,
            tc.tile_pool(name="psmall", bufs=4, space="PSUM") as psmall,
        ):
            # ---- consolidated input loads (big DMAs, two queues) ----
            def load(shape, dtype, src, tag, eng):
                t = const.tile(shape, dtype, name=tag, tag=tag)
                eng.dma_start(out=t, in_=src)
                return t

            r3 = lambda a: a.rearrange("(a p) n -> p a n", p=P)  # noqa: E731
            xT_a = load([P, KT, T], f16, r3(xT), "xT", nc.sync)
            wq_a = load([P, KT, C], f16, r3(wqT), "wq", nc.sync)
            wk_a = load([P, KT, C], f16, r3(wkT), "wk", nc.sync)
            wv_a = load([P, KT, C], f16, r3(wvT), "wv", nc.gpsimd)
            wp_a = load([P, KT, C], f16, r3(wpT), "wp", nc.gpsimd)
            bq_s = load([P, C // P], f32, bq, "bq", nc.gpsimd)
            bppb_s = load([P, C], f32, bppb, "bppb", nc.gpsimd)
            ones64_s = load([P, HD], f16, ones64, "ones64", nc.gpsimd)
            tri_s = load([P, P], f16, tri, "tri", nc.sync)
            cmask_s = load([P, T // P], f32, cmask, "cmask", nc.sync)

            qT_t = [persist.tile([P, T], f16, name=f"qT{m}", tag=f"qT{m}") for m in range(MT)]
            kT_t = [persist.tile([P, T], f16, name=f"kT{m}", tag=f"kT{m}") for m in range(MT)]
            v_t = [persist.tile([P, C], f16, name=f"v{r}", tag=f"v{r}") for r in range(RT)]
            yn_t = [persist.tile([P, T], f16, name=f"yn{m}", tag=f"yn{m}") for m in range(MT)]

            # broadcast lower-triangle tile across both heads of an es tile
            tri_b = bass.AP(
                tensor=tri_s.tensor,
                offset=tri_s.offset,
                ap=[list(tri_s.ap[0]), [0, 2], list(tri_s.ap[1])],
            )

            # ---- projections (128x128 mode) ----
            # q^T / k^T: out[co, t] = sum_c W^T[c, co] * xT[c, t]
            for m in range(MT):
                for w_a, dst, biased in ((wq_a, qT_t[m], True), (wk_a, kT_t[m], False)):
                    ps = pbig.tile([P, T], f32, name="psqk", tag="pbig")
                    for half in range(2):
                        o = ps[:, 512 * half : 512 * (half + 1)]
                        for k in range(KT):
                            nc.tensor.matmul(
                                o,
                                lhsT=w_a[:, k, P * m : P * (m + 1)],
                                rhs=xT_a[:, k, 512 * half : 512 * (half + 1)],
                                start=(k == 0),
                                stop=(k == KT - 1),
                            )
                    if biased:
                        nc.vector.tensor_scalar_add(dst, ps, bq_s[:, m : m + 1])
                    else:
                        nc.scalar.activation(dst, ps, AF.Copy)
            # v: out[t, co] = sum_c xT[c, t] * wvT[c, co]
            for r in range(RT):
                ps = psmall.tile([P, C], f32, name="pv", tag="sm")
                for k in range(KT):
                    nc.tensor.matmul(
                        ps,
                        lhsT=xT_a[:, k, P * r : P * (r + 1)],
                        rhs=wv_a[:, k, :],
                        start=(k == 0),
                        stop=(k == KT - 1),
                    )
                nc.scalar.activation(v_t[r], ps, AF.Copy)

            # ---- attention (phase-separated per i-chunk) ----
            for ic in range(2):
                Js = list(range(4 * (ic + 1)))
                es_t = {}
                # QK + exp phase (64x128 row tiling)
                for p in range(MT):
                    for J in Js:
                        i0 = max(512 * ic, P * J)
                        w = 512 * (ic + 1) - i0
                        st = pbig.tile([P, 2, 512], f32, name="st", tag="pbig")
                        for h in range(2):
                            nc.tensor.matmul(
                                st[:, h, :w],
                                lhsT=kT_t[p][64 * h : 64 * (h + 1), P * J : P * (J + 1)],
                                rhs=qT_t[p][64 * h : 64 * (h + 1), i0 : i0 + w],
                                start=True,
                                stop=True,
                                tile_position=(64 * h, 0),
                            )
                        es = es_pool.tile([P, 2, 512], f16, name="es", tag="es")
                        es_t[(p, J)] = es
                        nc.scalar.activation(
                            es[:, :, :w],
                            st[:, :, :w],
                            AF.Exp,
                            bias=cmask_s[:, J : J + 1],
                            scale=0.125,
                        )
                        if P * J >= 512 * ic:  # diagonal: zero the causal triangle
                            nc.vector.tensor_tensor(
                                out=es[:, :, :P],
                                in0=es[:, :, :P],
                                in1=tri_b,
                                op=ALU.mult,
                            )
                # AV + den phase (128x64 col tiling)
                for p in range(MT):
                    av = psmall.tile([P, 512], f32, name="av", tag="sm")
                    den = psmall.tile([P, 512], f32, name="den", tag="sm")
                    for idx, J in enumerate(Js):
                        i0 = max(512 * ic, P * J)
                        w = 512 * (ic + 1) - i0
                        io = i0 - 512 * ic
                        first, last = idx == 0, idx == len(Js) - 1
                        es = es_t[(p, J)]
                        for h in range(2):
                            nc.tensor.matmul(
                                av[64 * h : 64 * (h + 1), io : io + w],
                                lhsT=v_t[J][:, P * p + 64 * h : P * p + 64 * (h + 1)],
                                rhs=es[:, h, :w],
                                start=first,
                                stop=last,
                                tile_position=(0, 64 * h),
                            )
                            nc.tensor.matmul(
                                den[64 * h : 64 * (h + 1), io : io + w],
                                lhsT=ones64_s,
                                rhs=es[:, h, :w],
                                start=first,
                                stop=last,
                                tile_position=(0, 64 * h),
                            )
                    rden = rden_pool.tile([P, 512], f32, name="rden", tag="rden")
                    nc.vector.reciprocal_approx_fast(out=rden, in_=den)
                    nc.vector.tensor_mul(
                        yn_t[p][:, 512 * ic : 512 * (ic + 1)], av, rden
                    )
                # ---- output projection for this i-chunk (128x128 mode) ----
                for r in range(4 * ic, 4 * (ic + 1)):
                    po = psmall.tile([P, C], f32, name="po", tag="sm")
                    for m in range(MT):
                        nc.tensor.matmul(
                            po,
                            lhsT=yn_t[m][:, P * r : P * (r + 1)],
                            rhs=wp_a[:, m, :],
                            start=(m == 0),
                            stop=(m == MT - 1),
                        )
                    ot = ot_pool.tile([P, C], f32, name="ot", tag="ot")
                    nc.vector.tensor_tensor(out=ot, in0=po, in1=bppb_s, op=ALU.add)
                    nc.sync.dma_start(out=out[P * r : P * (r + 1), :], in_=ot)

    nc.compile()
    return nc


def _prep_inputs(x, Wq, bq, Wk, bk, Wv, bv, Wp, bp):
    """Host-side prep: transposes, bias folding, mask tables. Returns in_maps."""
    f16 = np.float16
    wqT = np.ascontiguousarray(Wq.T).astype(f16)
    wkT = np.ascontiguousarray(Wk.T).astype(f16)
    wvT = np.ascontiguousarray(Wv.T).astype(f16)
    wpT = np.ascontiguousarray(Wp.T).astype(f16)
    bq_pp = np.ascontiguousarray(bq.astype(np.float32).reshape(C // P, P).T)
    # v bias folds into output bias: out = (y' + bv) @ Wp.T + bp
    bpp = (
        Wp.astype(np.float64) @ bv.astype(np.float64) + bp.astype(np.float64)
    ).astype(np.float32)
    bppb = np.broadcast_to(bpp[None, :], (P, C)).copy()
    ones64 = np.ones((P, HD), dtype=f16)
    tri = (np.arange(P)[:, None] <= np.arange(P)[None, :]).astype(f16)  # keep j<=i
    j_idx = np.arange(P)[:, None] + P * np.arange(T // P)[None, :]
    cmask = np.where(j_idx % JD == JD - 1, np.float32(NEG), np.float32(0.0)).astype(
        np.float32
    )

    shared = {
        "wqT": wqT,
        "wkT": wkT,
        "wvT": wvT,
        "wpT": wpT,
        "bq": bq_pp,
        "bppb": bppb,
        "ones64": ones64,
        "tri": tri,
        "cmask": cmask,
    }
    in_maps = []
    for b in range(N_CORES):
        m = dict(shared)
        m["xT"] = np.ascontiguousarray(x[b].T).astype(f16)
        in_maps.append(m)
    return in_maps


def kernel(x, Wq, bq, Wk, bk, Wv, bv, Wp, bp):
    from concourse import bass_utils

    x = np.asarray(x, dtype=np.float32)
    if "nc" not in _CACHE:
        _CACHE["nc"] = _build()
    nc = _CACHE["nc"]
    in_maps = _prep_inputs(
        x,
        np.asarray(Wq, np.float32),
        np.asarray(bq, np.float32),
        np.asarray(Wk, np.float32),
        np.asarray(bk, np.float32),
        np.asarray(Wv, np.float32),
        np.asarray(bv, np.float32),
        np.asarray(Wp, np.float32),
        np.asarray(bp, np.float32),
    )
    res = bass_utils.run_bass_kernel_spmd(nc, in_maps, core_ids=list(range(N_CORES)))
    return np.stack([res.results[b]["out"] for b in range(N_CORES)], axis=0)


# revision 9
# speedup vs baseline: 1.1908x; 1.0042x over previous
"""Causal self-attention (sparse column mask) on 8 Trainium2 NeuronCores.

Problem: B=8, T=1024, C=512, 8 heads (hd=64).
  q/k/v = x @ W{q,k,v}.T + b;  att = softmax(mask(q k^T / 8));  y = att v
  out = y @ Wp.T + bp
Mask: causal lower-triangle, minus every column j with j % 25 == 24.

Sharding: pure data-parallel over batch — core b computes batch element b.

Per-core kernel design (all matmul operands fp16, PSUM accumulation f32):
  - Host pre-transposes x[b] -> xT [C, T] and all weights -> W^T [c_in, c_out],
    so every on-chip matmul has its contraction dim on partitions.
  - Projections produce q^T, k^T [C, T] (heads = partition blocks of 64) and
    v [T, C]. q bias is added during PSUM evacuation (DVE tensor_scalar,
    per-partition); k bias is dropped (softmax shift invariance); v bias is
    folded into the output bias on host (bp' = Wp @ bv + bp, sent broadcast).
  - Attention, phase-separated per query chunk ic (512 wide) so the PE array
    stays in one tiling mode per phase (mode switches drain the PE):
      QK phase (64x128 row-tiled): per head-pair p, per key tile J: two K=64
      matmuls (tile_position (0,0)/(64,0)) -> S^T in 2 PSUM banks; one ACT
      exp call over both (scale=1/8, per-partition bias -30 on j%25==24
      columns) -> fp16 SBUF; causal diagonal zeroed by one fp16 multiply with
      a broadcast lower-triangle tile on DVE.
      AV phase (128x64 col-tiled): per pair, accumulate y'^T and the
      replicated denominators (ones-weight matmuls) over J; then rden via
      approx reciprocal and one tensor_tensor multiply PSUM->SBUF fp16.
  - Output projection consumes y_norm^T directly; bias added during the DVE
    evacuation (tensor_tensor add with a host-broadcast bias tile).
"""

import numpy as np

B, T, C = 8, 1024, 512
H = 8
HD = C // H
P = 128
JD = 25  # joined dim; column j masked when j % 25 == 24
N_CORES = 8
NEG = -30.0  # added post-scale; exp(-30) flushes to 0 in fp16

_CACHE = {}


def _build():
    import concourse.bass as bass
    import concourse.mybir as mybir
    import concourse.tile as tile
    from concourse import bacc

    f16 = mybir.dt.float16
    f32 = mybir.dt.float32
    AF = mybir.ActivationFunctionType
    ALU = mybir.AluOpType

    nc = bacc.Bacc("TRN2", target_bir_lowering=False, debug=False)

    xT = nc.dram_tensor("xT", [C, T], f16, kind="ExternalInput").ap()
    wqT = nc.dram_tensor("wqT", [C, C], f16, kind="ExternalInput").ap()
    wkT = nc.dram_tensor("wkT", [C, C], f16, kind="ExternalInput").ap()
    wvT = nc.dram_tensor("wvT", [C, C], f16, kind="ExternalInput").ap()
    wpT = nc.dram_tensor("wpT", [C, C], f16, kind="ExternalInput").ap()
    bq = nc.dram_tensor("bq", [P, C // P], f32, kind="ExternalInput").ap()
    bppb = nc.dram_tensor("bppb", [P, C], f32, kind="ExternalInput").ap()
    ones64 = nc.dram_tensor("ones64", [P, HD], f16, kind="ExternalInput").ap()
    tri = nc.dram_tensor("tri", [P, P], f16, kind="ExternalInput").ap()
    cmask = nc.dram_tensor("cmask", [P, T // P], f32, kind="ExternalInput").ap()
    out = nc.dram_tensor("out", [T, C], f32, kind="ExternalOutput").ap()

    KT = C // P  # 4 c_in tiles
    MT = C // P  # 4 c_out tiles (= head pairs)
    RT = T // P  # 8 t tiles

    with tile.TileContext(nc) as tc:
        with (
            tc.tile_pool(name="const", bufs=1) as const,
            tc.tile_pool(name="persist", bufs=1) as persist,
            tc.tile_pool(name="es", bufs=16) as es_pool,
            tc.tile_pool(name="rden", bufs=2) as rden_pool,
            tc.tile_pool(name="ot", bufs=3) as ot_pool,
            tc.tile_pool(name="pbig", bufs=2, space="PSUM") as pbig,
            tc.tile_pool(name="psmall", bufs=4, space="PSUM") as psmall,
        ):
            # ---- consolidated input loads (big DMAs, two queues) ----
            def load(shape, dtype, src, tag, eng):
                t = const.tile(shape, dtype, name=tag, tag=tag)
                eng.dma_start(out=t, in_=src)
                return t

            r3 = lambda a: a.rearrange("(a p) n -> p a n", p=P)  # noqa: E731
            xT_a = load([P, KT, T], f16, r3(xT), "xT", nc.sync)
            wq_a = load([P, KT, C], f16, r3(wqT), "wq", nc.scalar)
            wk_a = load([P, KT, C], f16, r3(wkT), "wk", nc.scalar)
            wv_a = load([P, KT, C], f16, r3(wvT), "wv", nc.sync)
            wp_a = load([P, KT, C], f16, r3(wpT), "wp", nc.scalar)
            bq_s = load([P, C // P], f32, bq, "bq", nc.scalar)
            bppb_s = load([P, C], f32, bppb, "bppb", nc.sync)
            ones64_s = load([P, HD], f16, ones64, "ones64", nc.scalar)
            tri_s = load([P, P], f16, tri, "tri", nc.sync)
            cmask_s = load([P, T // P], f32, cmask, "cmask", nc.sync)

            qT_t = [persist.tile([P, T], f16, name=f"qT{m}", tag=f"qT{m}") for m in range(MT)]
            kT_t = [persist.tile([P, T], f16, name=f"kT{m}", tag=f"kT{m}") for m in range(MT)]
            v_t = [persist.tile([P, C], f16, name=f"v{r}", tag=f"v{r}") for r in range(RT)]
            yn_t = [persist.tile([P, T], f16, name=f"yn{m}", tag=f"yn{m}") for m in range(MT)]

            # broadcast lower-triangle tile across both heads of an es tile
            tri_b = bass.AP(
                tensor=tri_s.tensor,
                offset=tri_s.offset,
                ap=[list(tri_s.ap[0]), [0, 2], list(tri_s.ap[1])],
            )

            # ---- projections (128x128 mode) ----
            # q^T / k^T: out[co, t] = sum_c W^T[c, co] * xT[c, t]
            for m in range(MT):
                for w_a, dst, biased in ((wq_a, qT_t[m], True), (wk_a, kT_t[m], False)):
                    ps = pbig.tile([P, T], f32, name="psqk", tag="pbig")
                    for half in range(2):
                        o = ps[:, 512 * half : 512 * (half + 1)]
                        for k in range(KT):
                            nc.tensor.matmul(
                                o,
                                lhsT=w_a[:, k, P * m : P * (m + 1)],
                                rhs=xT_a[:, k, 512 * half : 512 * (half + 1)],
                                start=(k == 0),
                                stop=(k == KT - 1),
                            )
                    if biased:
                        nc.vector.tensor_scalar_add(dst, ps, bq_s[:, m : m + 1])
                    else:
                        nc.scalar.activation(dst, ps, AF.Copy)
            # v: out[t, co] = sum_c xT[c, t] * wvT[c, co]
            for r in range(RT):
                ps = psmall.tile([P, C], f32, name="pv", tag="sm")
                for k in range(KT):
                    nc.tensor.matmul(
                        ps,
                        lhsT=xT_a[:, k, P * r : P * (r + 1)],
                        rhs=wv_a[:, k, :],
                        start=(k == 0),
                        stop=(k == KT - 1),
                    )
                nc.scalar.activation(v_t[r], ps, AF.Copy)

            # ---- attention: software-pipelined pairs per i-chunk ----
            # QK+exp for pair p overlaps AV+den (different PE tiling mode)
            # for pair p-1; es tiles double-buffered across two pairs.
            for ic in range(2):
                Js = list(range(4 * (ic + 1)))
                es_t = {}

                def qk_phase(p, ic=ic, Js=Js, es_t=es_t):
                    for J in Js:
                        i0 = max(512 * ic, P * J)
                        w = 512 * (ic + 1) - i0
                        st = pbig.tile([P, 2, 512], f32, name="st", tag="pbig")
                        for h in range(2):
                            nc.tensor.matmul(
                                st[:, h, :w],
                                lhsT=kT_t[p][64 * h : 64 * (h + 1), P * J : P * (J + 1)],
                                rhs=qT_t[p][64 * h : 64 * (h + 1), i0 : i0 + w],
                                start=True,
                                stop=True,
                                tile_position=(64 * h, 0),
                            )
                        es = es_pool.tile([P, 2, 512], f16, name="es", tag="es")
                        es_t[(p, J)] = es
                        nc.scalar.activation(
                            es[:, :, :w],
                            st[:, :, :w],
                            AF.Exp,
                            bias=cmask_s[:, J : J + 1],
                            scale=0.125,
                        )
                        if P * J >= 512 * ic:  # diagonal: zero the causal triangle
                            nc.vector.tensor_tensor(
                                out=es[:, :, :P],
                                in0=es[:, :, :P],
                                in1=tri_b,
                                op=ALU.mult,
                            )

                def av_phase(p, ic=ic, Js=Js, es_t=es_t):
                    av = psmall.tile([P, 512], f32, name="av", tag="sm")
                    den = psmall.tile([P, 512], f32, name="den", tag="sm")
                    for idx, J in enumerate(Js):
                        i0 = max(512 * ic, P * J)
                        w = 512 * (ic + 1) - i0
                        io = i0 - 512 * ic
                        first, last = idx == 0, idx == len(Js) - 1
                        es = es_t[(p, J)]
                        for h in range(2):
                            nc.tensor.matmul(
                                av[64 * h : 64 * (h + 1), io : io + w],
                                lhsT=v_t[J][:, P * p + 64 * h : P * p + 64 * (h + 1)],
                                rhs=es[:, h, :w],
                                start=first,
                                stop=last,
                                tile_position=(0, 64 * h),
                            )
                            nc.tensor.matmul(
                                den[64 * h : 64 * (h + 1), io : io + w],
                                lhsT=ones64_s,
                                rhs=es[:, h, :w],
                                start=first,
                                stop=last,
                                tile_position=(0, 64 * h),
                            )
                    rden = rden_pool.tile([P, 512], f32, name="rden", tag="rden")
                    nc.vector.reciprocal_approx_fast(out=rden, in_=den)
                    nc.vector.tensor_mul(
                        yn_t[p][:, 512 * ic : 512 * (ic + 1)], av, rden
                    )

                for p in range(MT):
                    qk_phase(p)
                    if p >= 1:
                        av_phase(p - 1)
                av_phase(MT - 1)
                # ---- output projection for this i-chunk (128x128 mode) ----
                for r in range(4 * ic, 4 * (ic + 1)):
                    po = psmall.tile([P, C], f32, name="po", tag="sm")
                    for m in range(MT):
                        nc.tensor.matmul(
                            po,
                            lhsT=yn_t[m][:, P * r : P * (r + 1)],
                            rhs=wp_a[:, m, :],
                            start=(m == 0),
                            stop=(m == MT - 1),
                        )
                    ot = ot_pool.tile([P, C], f32, name="ot", tag="ot")
                    nc.vector.tensor_tensor(out=ot, in0=po, in1=bppb_s, op=ALU.add)
                    nc.sync.dma_start(out=out[P * r : P * (r + 1), :], in_=ot)

    nc.compile()
    return nc


def _prep_inputs(x, Wq, bq, Wk, bk, Wv, bv, Wp, bp):
    """Host-side prep: transposes, bias folding, mask tables. Returns in_maps."""
    f16 = np.float16
    wqT = np.ascontiguousarray(Wq.T).astype(f16)
    wkT = np.ascontiguousarray(Wk.T).astype(f16)
    wvT = np.ascontiguousarray(Wv.T).astype(f16)
    wpT = np.ascontiguousarray(Wp.T).astype(f16)
    bq_pp = np.ascontiguousarray(bq.astype(np.float32).reshape(C // P, P).T)
    # v bias folds into output bias: out = (y' + bv) @ Wp.T + bp
    bpp = (
        Wp.astype(np.float64) @ bv.astype(np.float64) + bp.astype(np.float64)
    ).astype(np.float32)
    bppb = np.broadcast_to(bpp[None, :], (P, C)).copy()
    ones64 = np.ones((P, HD), dtype=f16)
    tri = (np.arange(P)[:, None] <= np.arange(P)[None, :]).astype(f16)  # keep j<=i
    j_idx = np.arange(P)[:, None] + P * np.arange(T // P)[None, :]
    cmask = np.where(j_idx % JD == JD - 1, np.float32(NEG), np.float32(0.0)).astype(
        np.float32
    )

    shared = {
        "wqT": wqT,
        "wkT": wkT,
        "wvT": wvT,
        "wpT": wpT,
        "bq": bq_pp,
        "bppb": bppb,
        "ones64": ones64,
        "tri": tri,
        "cmask": cmask,
    }
    in_maps = []
    for b in range(N_CORES):
        m = dict(shared)
        m["xT"] = np.ascontiguousarray(x[b].T).astype(f16)
        in_maps.append(m)
    return in_maps


def kernel(x, Wq, bq, Wk, bk, Wv, bv, Wp, bp):
    from concourse import bass_utils

    x = np.asarray(x, dtype=np.float32)
    if "nc" not in _CACHE:
        _CACHE["nc"] = _build()
    nc = _CACHE["nc"]
    in_maps = _prep_inputs(
        x,
        np.asarray(Wq, np.float32),
        np.asarray(bq, np.float32),
        np.asarray(Wk, np.float32),
        np.asarray(bk, np.float32),
        np.asarray(Wv, np.float32),
        np.asarray(bv, np.float32),
        np.asarray(Wp, np.float32),
        np.asarray(bp, np.float32),
    )
    res = bass_utils.run_bass_kernel_spmd(nc, in_maps, core_ids=list(range(N_CORES)))
    return np.stack([res.results[b]["out"] for b in range(N_CORES)], axis=0)


# revision 10
# speedup vs baseline: 1.2092x; 1.0154x over previous
"""Causal self-attention (sparse column mask) on 8 Trainium2 NeuronCores.

Problem: B=8, T=1024, C=512, 8 heads (hd=64).
  q/k/v = x @ W{q,k,v}.T + b;  att = softmax(mask(q k^T / 8));  y = att v
  out = y @ Wp.T + bp
Mask: causal lower-triangle, minus every column j with j % 25 == 24.

Sharding: pure data-parallel over batch — core b computes batch element b.

Per-core kernel design (all matmul operands fp16, PSUM accumulation f32):
  - Host pre-transposes x[b] -> xT [C, T] and all weights -> W^T [c_in, c_out],
    so every on-chip matmul has its contraction dim on partitions.
  - Projections produce q^T, k^T [C, T] (heads = partition blocks of 64) and
    v [T, C]. q bias is added during PSUM evacuation (DVE tensor_scalar,
    per-partition); k bias is dropped (softmax shift invariance); v bias is
    folded into the output bias on host (bp' = Wp @ bv + bp, sent broadcast).
  - Attention, phase-separated per query chunk ic (512 wide) so the PE array
    stays in one tiling mode per phase (mode switches drain the PE):
      QK phase (64x128 row-tiled): per head-pair p, per key tile J: two K=64
      matmuls (tile_position (0,0)/(64,0)) -> S^T in 2 PSUM banks; one ACT
      exp call over both (scale=1/8, per-partition bias -30 on j%25==24
      columns) -> fp16 SBUF; causal diagonal zeroed by one fp16 multiply with
      a broadcast lower-triangle tile on DVE.
      AV phase (128x64 col-tiled): per pair, accumulate y'^T and the
      replicated denominators (ones-weight matmuls) over J; then rden via
      approx reciprocal and one tensor_tensor multiply PSUM->SBUF fp16.
  - Output projection consumes y_norm^T directly; bias added during the DVE
    evacuation (tensor_tensor add with a host-broadcast bias tile).
"""

import numpy as np

B, T, C = 8, 1024, 512
H = 8
HD = C // H
P = 128
JD = 25  # joined dim; column j masked when j % 25 == 24
N_CORES = 8
NEG = -30.0  # added post-scale; exp(-30) flushes to 0 in fp16

_CACHE = {}


def _build():
    import concourse.bass as bass
    import concourse.mybir as mybir
    import concourse.tile as tile
    from concourse import bacc

    f16 = mybir.dt.float16
    f32 = mybir.dt.float32
    AF = mybir.ActivationFunctionType
    ALU = mybir.AluOpType

    nc = bacc.Bacc("TRN2", target_bir_lowering=False, debug=False)

    xT = nc.dram_tensor("xT", [C, T], f16, kind="ExternalInput").ap()
    wqT = nc.dram_tensor("wqT", [C, C], f16, kind="ExternalInput").ap()
    wkT = nc.dram_tensor("wkT", [C, C], f16, kind="ExternalInput").ap()
    wvT = nc.dram_tensor("wvT", [C, C], f16, kind="ExternalInput").ap()
    wpT = nc.dram_tensor("wpT", [C, C], f16, kind="ExternalInput").ap()
    bq = nc.dram_tensor("bq", [P, C // P], f32, kind="ExternalInput").ap()
    bppb = nc.dram_tensor("bppb", [P, C], f32, kind="ExternalInput").ap()
    ones64 = nc.dram_tensor("ones64", [P, HD], f16, kind="ExternalInput").ap()
    tri = nc.dram_tensor("tri", [P, P], f16, kind="ExternalInput").ap()
    cmask = nc.dram_tensor("cmask", [P, T // P], f32, kind="ExternalInput").ap()
    out = nc.dram_tensor("out", [T, C], f32, kind="ExternalOutput").ap()

    KT = C // P  # 4 c_in tiles
    MT = C // P  # 4 c_out tiles (= head pairs)
    RT = T // P  # 8 t tiles

    with tile.TileContext(nc) as tc:
        with (
            tc.tile_pool(name="const", bufs=1) as const,
            tc.tile_pool(name="persist", bufs=1) as persist,
            tc.tile_pool(name="es", bufs=16) as es_pool,
            tc.tile_pool(name="rden", bufs=2) as rden_pool,
            tc.tile_pool(name="ot", bufs=3) as ot_pool,
            tc.tile_pool(name="pbig", bufs=2, space="PSUM") as pbig,
            tc.tile_pool(name="psmall", bufs=4, space="PSUM") as psmall,
        ):
            # ---- consolidated input loads (big DMAs, two queues) ----
            def load(shape, dtype, src, tag, eng):
                t = const.tile(shape, dtype, name=tag, tag=tag)
                eng.dma_start(out=t, in_=src)
                return t

            r3 = lambda a: a.rearrange("(a p) n -> p a n", p=P)  # noqa: E731
            # first-needed data in small chunks so the first matmuls start early
            xT_lo = load([P, 2, T], f16, r3(xT)[:, 0:2, :], "xTlo", nc.sync)
            wq_lo = load([P, 2, C], f16, r3(wqT)[:, 0:2, :], "wqlo", nc.scalar)
            xT_hi = load([P, 2, T], f16, r3(xT)[:, 2:4, :], "xThi", nc.sync)
            wq_hi = load([P, 2, C], f16, r3(wqT)[:, 2:4, :], "wqhi", nc.scalar)
            wk_a = load([P, KT, C], f16, r3(wkT), "wk", nc.scalar)
            wv_a = load([P, KT, C], f16, r3(wvT), "wv", nc.sync)
            wp_a = load([P, KT, C], f16, r3(wpT), "wp", nc.scalar)
            bq_s = load([P, C // P], f32, bq, "bq", nc.scalar)
            bppb_s = load([P, C], f32, bppb, "bppb", nc.sync)
            ones64_s = load([P, HD], f16, ones64, "ones64", nc.scalar)
            tri_s = load([P, P], f16, tri, "tri", nc.sync)
            cmask_s = load([P, T // P], f32, cmask, "cmask", nc.sync)

            def xt(k):
                return xT_lo[:, k, :] if k < 2 else xT_hi[:, k - 2, :]

            def wq(k):
                return wq_lo[:, k, :] if k < 2 else wq_hi[:, k - 2, :]

            qT_t = [persist.tile([P, T], f16, name=f"qT{m}", tag=f"qT{m}") for m in range(MT)]
            kT_t = [persist.tile([P, T], f16, name=f"kT{m}", tag=f"kT{m}") for m in range(MT)]
            v_t = [persist.tile([P, C], f16, name=f"v{r}", tag=f"v{r}") for r in range(RT)]
            yn_t = [persist.tile([P, T], f16, name=f"yn{m}", tag=f"yn{m}") for m in range(MT)]

            # broadcast lower-triangle tile across both heads of an es tile
            tri_b = bass.AP(
                tensor=tri_s.tensor,
                offset=tri_s.offset,
                ap=[list(tri_s.ap[0]), [0, 2], list(tri_s.ap[1])],
            )

            # ---- projections (128x128 mode) ----
            # q^T / k^T: out[co, t] = sum_c W^T[c, co] * xT[c, t]
            for m in range(MT):
                for which, dst, biased in (("q", qT_t[m], True), ("k", kT_t[m], False)):
                    ps = pbig.tile([P, T], f32, name="psqk", tag="pbig")
                    for half in range(2):
                        o = ps[:, 512 * half : 512 * (half + 1)]
                        for k in range(KT):
                            w_ap = wq(k)[:, P * m : P * (m + 1)] if which == "q" else wk_a[:, k, P * m : P * (m + 1)]
                            nc.tensor.matmul(
                                o,
                                lhsT=w_ap,
                                rhs=xt(k)[:, 512 * half : 512 * (half + 1)],
                                start=(k == 0),
                                stop=(k == KT - 1),
                            )
                    if biased:
                        nc.vector.tensor_scalar_add(dst, ps, bq_s[:, m : m + 1])
                    else:
                        nc.scalar.activation(dst, ps, AF.Copy)
            # v: out[t, co] = sum_c xT[c, t] * wvT[c, co]
            for r in range(RT):
                ps = psmall.tile([P, C], f32, name="pv", tag="sm")
                for k in range(KT):
                    nc.tensor.matmul(
                        ps,
                        lhsT=xt(k)[:, P * r : P * (r + 1)],
                        rhs=wv_a[:, k, :],
                        start=(k == 0),
                        stop=(k == KT - 1),
                    )
                nc.scalar.activation(v_t[r], ps, AF.Copy)

            # ---- attention: software-pipelined pairs per i-chunk ----
            # QK+exp for pair p overlaps AV+den (different PE tiling mode)
            # for pair p-1; es tiles double-buffered across two pairs.
            for ic in range(2):
                Js = list(range(4 * (ic + 1)))
                es_t = {}

                def qk_phase(p, ic=ic, Js=Js, es_t=es_t):
                    for J in Js:
                        i0 = max(512 * ic, P * J)
                        w = 512 * (ic + 1) - i0
                        st = pbig.tile([P, 2, 512], f32, name="st", tag="pbig")
                        for h in range(2):
                            nc.tensor.matmul(
                                st[:, h, :w],
                                lhsT=kT_t[p][64 * h : 64 * (h + 1), P * J : P * (J + 1)],
                                rhs=qT_t[p][64 * h : 64 * (h + 1), i0 : i0 + w],
                                start=True,
                                stop=True,
                                tile_position=(64 * h, 0),
                            )
                        es = es_pool.tile([P, 2, 512], f16, name="es", tag="es")
                        es_t[(p, J)] = es
                        nc.scalar.activation(
                            es[:, :, :w],
                            st[:, :, :w],
                            AF.Exp,
                            bias=cmask_s[:, J : J + 1],
                            scale=0.125,
                        )
                        if P * J >= 512 * ic:  # diagonal: zero the causal triangle
                            nc.vector.tensor_tensor(
                                out=es[:, :, :P],
                                in0=es[:, :, :P],
                                in1=tri_b,
                                op=ALU.mult,
                            )

                def av_phase(p, ic=ic, Js=Js, es_t=es_t):
                    av = psmall.tile([P, 512], f32, name="av", tag="sm")
                    den = psmall.tile([P, 512], f32, name="den", tag="sm")
                    for idx, J in enumerate(Js):
                        i0 = max(512 * ic, P * J)
                        w = 512 * (ic + 1) - i0
                        io = i0 - 512 * ic
                        first, last = idx == 0, idx == len(Js) - 1
                        es = es_t[(p, J)]
                        for h in range(2):
                            nc.tensor.matmul(
                                av[64 * h : 64 * (h + 1), io : io + w],
                                lhsT=v_t[J][:, P * p + 64 * h : P * p + 64 * (h + 1)],
                                rhs=es[:, h, :w],
                                start=first,
                                stop=last,
                                tile_position=(0, 64 * h),
                            )
                            nc.tensor.matmul(
                                den[64 * h : 64 * (h + 1), io : io + w],
                                lhsT=ones64_s,
                                rhs=es[:, h, :w],
                                start=first,
                                stop=last,
                                tile_position=(0, 64 * h),
                            )
                    rden = rden_pool.tile([P, 512], f32, name="rden", tag="rden")
                    nc.vector.reciprocal_approx_fast(out=rden, in_=den)
                    nc.vector.tensor_mul(
                        yn_t[p][:, 512 * ic : 512 * (ic + 1)], av, rden
                    )

                for p in range(MT):
                    qk_phase(p)
                    if p >= 1:
                        av_phase(p - 1)
                av_phase(MT - 1)
                # ---- output projection for this i-chunk (128x128 mode) ----
                for r in range(4 * ic, 4 * (ic + 1)):
                    po = psmall.tile([P, C], f32, name="po", tag="sm")
                    for m in range(MT):
                        nc.tensor.matmul(
                            po,
                            lhsT=yn_t[m][:, P * r : P * (r + 1)],
                            rhs=wp_a[:, m, :],
                            start=(m == 0),
                            stop=(m == MT - 1),
                        )
                    ot = ot_pool.tile([P, C], f32, name="ot", tag="ot")
                    nc.vector.tensor_tensor(out=ot, in0=po, in1=bppb_s, op=ALU.add)
                    nc.sync.dma_start(out=out[P * r : P * (r + 1), :], in_=ot)

    nc.compile()
    return nc


def _prep_inputs(x, Wq, bq, Wk, bk, Wv, bv, Wp, bp):
    """Host-side prep: transposes, bias folding, mask tables. Returns in_maps."""
    f16 = np.float16
    wqT = np.ascontiguousarray(Wq.T).astype(f16)
    wkT = np.ascontiguousarray(Wk.T).astype(f16)
    wvT = np.ascontiguousarray(Wv.T).astype(f16)
    wpT = np.ascontiguousarray(Wp.T).astype(f16)
    bq_pp = np.ascontiguousarray(bq.astype(np.float32).reshape(C // P, P).T)
    # v bias folds into output bias: out = (y' + bv) @ Wp.T + bp
    bpp = (
        Wp.astype(np.float64) @ bv.astype(np.float64) + bp.astype(np.float64)
    ).astype(np.float32)
    bppb = np.broadcast_to(bpp[None, :], (P, C)).copy()
    ones64 = np.ones((P, HD), dtype=f16)
    tri = (np.arange(P)[:, None] <= np.arange(P)[None, :]).astype(f16)  # keep j<=i
    j_idx = np.arange(P)[:, None] + P * np.arange(T // P)[None, :]
    cmask = np.where(j_idx % JD == JD - 1, np.float32(NEG), np.float32(0.0)).astype(
        np.float32
    )

    shared = {
        "wqT": wqT,
        "wkT": wkT,
        "wvT": wvT,
        "wpT": wpT,
        "bq": bq_pp,
        "bppb": bppb,
        "ones64": ones64,
        "tri": tri,
        "cmask": cmask,
    }
    in_maps = []
    for b in range(N_CORES):
        m = dict(shared)
        m["xT"] = np.ascontiguousarray(x[b].T).astype(f16)
        in_maps.append(m)
    return in_maps


def kernel(x, Wq, bq, Wk, bk, Wv, bv, Wp, bp):
    from concourse import bass_utils

    x = np.asarray(x, dtype=np.float32)
    if "nc" not in _CACHE:
        _CACHE["nc"] = _build()
    nc = _CACHE["nc"]
    in_maps = _prep_inputs(
        x,
        np.asarray(Wq, np.float32),
        np.asarray(bq, np.float32),
        np.asarray(Wk, np.float32),
        np.asarray(bk, np.float32),
        np.asarray(Wv, np.float32),
        np.asarray(bv, np.float32),
        np.asarray(Wp, np.float32),
        np.asarray(bp, np.float32),
    )
    res = bass_utils.run_bass_kernel_spmd(nc, in_maps, core_ids=list(range(N_CORES)))
    return np.stack([res.results[b]["out"] for b in range(N_CORES)], axis=0)
